# revision 1
# baseline (speedup 1.0000x reference)
"""ConvDownsample2D (StyleGAN2 FIR blur + strided conv) for 8 Trainium2 cores.

Sharding: data-parallel over batch, 1 image per NeuronCore.

Per-core pipeline (all compute in fp16 with fp32 PSUM accumulation):
  1. x is passed host-relaid as [W, H, C] fp16 so input DMA descriptors are
     large and contiguous.
  2. H-blur runs ON THE TENSOR ENGINE as a banded matmul y_h = x_wtile.T @ B
     (contraction over image columns) which also transposes NHWC into
     channel-major layout for free.
  3. V-blur runs on the vector engine. For blur kernels that factor into
     unit-coefficient 2-taps (e.g. [1,3,3,1] = [1,1]^3) it is a cascade of
     plain adds (DVE 2x mode); otherwise a scalar_tensor_tensor tap chain.
  4. The 3x3/stride-2 conv is 9 accumulating matmuls per output row
     (lhsT = blurred activations [C,128pix], rhs = W taps [C,256oc]) plus a
     K=1 matmul adding the bias. PSUM banks hold 2 output rows.
"""
import sys

if "/opt/trn_rl_repo" not in sys.path:
    sys.path.insert(0, "/opt/trn_rl_repo")

import numpy as np

import concourse.bass as bass
import concourse.tile as tile
from concourse import bacc, mybir
from concourse.bass_utils import run_bass_kernel_spmd

F16 = mybir.dt.float16
F32 = mybir.dt.float32

N_CORES = 8
H = W = 256
C = 128
OC = 256
OH = OW = 128
WP = W + 1          # 257 blurred width
PITCH = 258         # even row pitch (fp16 4B alignment for DVE 2x mode)
P_BLK = 16          # output rows per block
N_BLK = OH // P_BLK
N_YH = 2 * P_BLK + 4   # 36 y_h rows per block incl halo
N_YV = 2 * P_BLK + 1   # 33 y_v rows per block
XGRP = 18              # y_h rows loaded per input DMA

# (p0, pblk) per block; small first block shortens the pipeline fill
import os as _os
_sizes = [int(v) for v in _os.environ.get(
    "KBLOCKS", "8,16,16,16,16,16,16,16,8").split(",")]
BLOCKS = []
_p = 0
for _s in _sizes:
    BLOCKS.append((_p, _s))
    _p += _s
assert _p == OH


def _build_bass(mode, repeat=1):
    nc = bacc.Bacc("TRN2", target_bir_lowering=False, debug=False)

    x16 = nc.dram_tensor("x16", [W, H, C], F16, kind="ExternalInput").ap()
    b_a = nc.dram_tensor("b_a", [128, 131], F16, kind="ExternalInput").ap()
    b_b = nc.dram_tensor("b_b", [128, 130], F16, kind="ExternalInput").ap()
    w16 = nc.dram_tensor("w16", [9, C, OC], F16, kind="ExternalInput").ap()
    ones = nc.dram_tensor("ones", [1, 128], F16, kind="ExternalInput").ap()
    bias = nc.dram_tensor("bias", [1, 2 * OC], F16, kind="ExternalInput").ap()
    if mode == "general":
        kvt = nc.dram_tensor("kvt", [128, 4], F32, kind="ExternalInput").ap()
    out = nc.dram_tensor("out", [OH, OW, OC], F32, kind="ExternalOutput").ap()

    if mode == "b1331":
        stage_shifts = [1, 1, 1]
    elif mode == "b1111":
        stage_shifts = [1, 2]
    else:
        stage_shifts = None

    with tile.TileContext(nc) as tc:
        with (
            tc.tile_pool(name="const", bufs=1) as cpool,
            tc.tile_pool(name="xin", bufs=6) as xpool,
            tc.tile_pool(name="yh", bufs=2) as yhpool,
            tc.tile_pool(name="yv", bufs=2) as yvpool,
            tc.tile_pool(name="osb", bufs=4) as opool,
            tc.tile_pool(name="pyh", bufs=2, space=bass.MemorySpace.PSUM) as pyh,
            tc.tile_pool(name="pout", bufs=4, space=bass.MemorySpace.PSUM) as pout,
        ):
            ba_sb = cpool.tile([128, 131], F16)
            nc.sync.dma_start(ba_sb[:], b_a[:])
            bb_sb = cpool.tile([128, 130], F16)
            nc.sync.dma_start(bb_sb[:], b_b[:])
            w_sb = cpool.tile([128, 9, OC], F16)
            ones_sb = cpool.tile([1, 128], F16)
            bias_sb = cpool.tile([1, 2 * OC], F16)
            kv_sb = cpool.tile([128, 4], F32) if mode == "general" else None

            def load_weights():
                # deferred so block-0 input DMAs go first on the DGE queue
                for t in range(9):
                    nc.sync.dma_start(w_sb[:, t, :], w16[t])
                nc.sync.dma_start(ones_sb[:], ones[:])
                nc.sync.dma_start(bias_sb[:], bias[:])
                if mode == "general":
                    nc.sync.dma_start(kv_sb[:], kvt[:])

            def blur_block(k):
                """Emit loads + H-blur (PE) + V-blur (DVE) for block k.
                Returns the block's y_v tile."""
                p0, pblk = BLOCKS[k]
                n_yh = 2 * pblk + 4
                n_yv = 2 * pblk + 1
                hs0 = 2 * p0 - 2          # first y_h row (may be <0 / >=H)
                s_lo = max(0, -hs0)       # first valid slot
                s_hi = min(n_yh, H - hs0)  # end of valid slots (both even)

                yh_t = yhpool.tile([128, N_YH, PITCH], F16, tag="yh")
                if s_lo > 0:
                    nc.gpsimd.memset(yh_t[:, 0:s_lo, :], 0.0)
                if s_hi < n_yh:
                    nc.gpsimd.memset(yh_t[:, s_hi:n_yh, :], 0.0)

                # ---- input loads: XGRP y_h rows per DMA, 2 w-halves ----
                xg = 9 if k == 0 else XGRP  # fine first loads: PE starts sooner
                xtiles = []
                for g0 in range(s_lo, s_hi, xg):
                    sz = min(xg, s_hi - g0)
                    pair = []
                    for t in range(2):
                        xt = xpool.tile([128, XGRP, C], F16, tag=f"x{t}")
                        nc.sync.dma_start(
                            xt[:, 0:sz, :],
                            x16[t * 128 : (t + 1) * 128, hs0 + g0 : hs0 + g0 + sz, :],
                        )
                        pair.append(xt)
                    xtiles.append((g0, pair))

                def xslice(s):
                    for g0, pair in xtiles:
                        if g0 <= s < g0 + xg:
                            return pair[0][:, s - g0, :], pair[1][:, s - g0, :]
                    raise AssertionError(s)

                # ---- H-blur (+ NHWC->channel-major transpose) on PE ----
                for ip, s0 in enumerate(range(s_lo, s_hi, 2)):
                    pp = pyh.tile([128, 2, 512], F32)
                    for e in range(2):
                        xlo, xhi = xslice(s0 + e)
                        nc.tensor.matmul(
                            pp[:, e, 0:131], xlo, ba_sb[:],
                            start=True, stop=False,
                        )
                        nc.tensor.matmul(
                            pp[:, e, 127:257], xhi, bb_sb[:],
                            start=False, stop=True, skip_group_check=True,
                        )
                    dst = yh_t[:, s0 : s0 + 2, 0:WP]
                    if ip % 2 == 0:
                        nc.scalar.copy(dst, pp[:, :, 0:WP])
                    else:
                        nc.vector.tensor_copy(dst, pp[:, :, 0:WP])

                # ---- V-blur on DVE ----
                yv_t = yvpool.tile([128, N_YV, PITCH], F16, tag="yv")
                if stage_shifts is not None:
                    n = n_yh
                    for i, sh in enumerate(stage_shifts):
                        n -= sh
                        dst = yv_t[:, 0:n, :] if i == len(stage_shifts) - 1 else yh_t[:, 0:n, :]
                        nc.vector.tensor_add(
                            dst, yh_t[:, 0:n, :], yh_t[:, sh : sh + n, :]
                        )
                    assert n == n_yv
                else:
                    nc.vector.tensor_scalar(
                        yv_t[:, 0:n_yv, :],
                        yh_t[:, 0:n_yv, :],
                        kv_sb[:, 0:1],
                        None,
                        mybir.AluOpType.mult,
                    )
                    for u in range(1, 4):
                        nc.vector.scalar_tensor_tensor(
                            yv_t[:, 0:n_yv, :],
                            yh_t[:, u : u + n_yv, :],
                            kv_sb[:, u : u + 1],
                            yv_t[:, 0:n_yv, :],
                            mybir.AluOpType.mult,
                            mybir.AluOpType.add,
                        )
                return yv_t

            def conv_block(k, yv_t):
                """Emit conv 3x3 stride-2 + bias (PE) + evac + store for block k."""
                p0, pblk = BLOCKS[k]
                for pr in range(pblk // 2):
                    po = pout.tile([128, 2, OC], F32)  # one bank: 2 out rows
                    for e in range(2):
                        r0 = 2 * (2 * pr + e)
                        for t in range(9):
                            a, b = divmod(t, 3)
                            lhsT = yv_t[:, r0 + a, b : b + 256 : 2]
                            nc.tensor.matmul(
                                po[:, e, :], lhsT, w_sb[:, t, :],
                                start=(e == 0 and t == 0), stop=False,
                                skip_group_check=True,
                            )
                    # bias for both rows in one K=1, N=512 matmul
                    nc.tensor.matmul(
                        po[:, :, :].rearrange("p a b -> p (a b)"),
                        ones_sb[:], bias_sb[:],
                        start=False, stop=True, skip_group_check=True,
                    )
                    ot = opool.tile([128, 2, OC], F32)
                    nc.scalar.copy(ot[:], po[:])
                    p = p0 + 2 * pr
                    dst = out[p : p + 2, :, :].rearrange("r q o -> q r o")
                    nc.sync.dma_start(dst, ot[:])

            # software pipeline: blur of block k+1 overlaps conv of block k
            for rep in range(repeat):
                prev = None
                for k in range(len(BLOCKS)):
                    yv_t = blur_block(k)
                    if k == 0 and rep == 0:
                        load_weights()
                    if prev is not None:
                        conv_block(k - 1, prev)
                    prev = yv_t
                conv_block(len(BLOCKS) - 1, prev)

    nc.compile()
    return nc


_NC = {}


def _get_nc(mode="b1331", repeat=1):
    key = (mode, repeat)
    if key not in _NC:
        _NC[key] = _build_bass(mode, repeat)
    return _NC[key]


def _blur_mode(bk):
    k8 = bk / bk.sum() * 8.0
    if np.allclose(k8, [1.0, 3.0, 3.0, 1.0], rtol=1e-6, atol=1e-7):
        return "b1331"
    k4 = bk / bk.sum() * 4.0
    if np.allclose(k4, [1.0, 1.0, 1.0, 1.0], rtol=1e-6, atol=1e-7):
        return "b1111"
    return "general"


def _prepare_in_maps(x, conv_w, conv_b, blur_kernel):
    x = np.asarray(x, dtype=np.float32)
    conv_w = np.asarray(conv_w, dtype=np.float32)
    conv_b = np.asarray(conv_b, dtype=np.float32)
    bk = np.asarray(blur_kernel, dtype=np.float32)

    mode = _blur_mode(bk)
    k1 = (bk / bk.sum()).astype(np.float32)  # separable normalized taps

    # banded H-blur matrices (normalization folded in)
    Bfull = np.zeros((W, WP), np.float32)
    j = np.arange(W)[:, None]
    wp = np.arange(WP)[None, :]
    d = j - wp + 2
    m = (d >= 0) & (d <= 3)
    Bfull[m] = k1[d[m]]
    b_a = Bfull[0:128, 0:131].astype(np.float16)
    b_b = Bfull[128:256, 127:257].astype(np.float16)

    # V-direction normalization: box cascades compute the UNNORMALIZED sum,
    # so fold 1/sum(bk) into the conv weights for those modes.
    wscale = 1.0 / bk.sum() if mode in ("b1331", "b1111") else 1.0
    w16 = np.ascontiguousarray(
        (conv_w * wscale).reshape(9, C, OC).astype(np.float16)
    )
    ones = np.ones((1, 128), np.float16)
    bias = np.concatenate([conv_b, conv_b]).reshape(1, 2 * OC).astype(np.float16)

    in_maps = []
    for i in range(N_CORES):
        im = {
            "x16": np.ascontiguousarray(
                x[i].transpose(1, 0, 2).astype(np.float16)
            ),
            "b_a": b_a,
            "b_b": b_b,
            "w16": w16,
            "ones": ones,
            "bias": bias,
        }
        if mode == "general":
            im["kvt"] = np.ascontiguousarray(
                np.broadcast_to(k1[None, :], (128, 4)).astype(np.float32)
            )
        in_maps.append(im)
    return mode, in_maps


def _run(mode, in_maps, **kwargs):
    nc = _get_nc(mode)
    return run_bass_kernel_spmd(nc, in_maps, core_ids=list(range(N_CORES)), **kwargs)


def kernel(x, conv_w, conv_b, blur_kernel):
    mode, in_maps = _prepare_in_maps(x, conv_w, conv_b, blur_kernel)
    res = _run(mode, in_maps)
    return np.stack([res.results[i]["out"] for i in range(N_CORES)], axis=0)



# revision 2
# speedup vs baseline: 1.1376x; 1.1376x over previous
"""ConvDownsample2D (StyleGAN2 FIR blur + strided conv) for 8 Trainium2 cores.

Sharding: data-parallel over batch, 1 image per NeuronCore.

v2 design — minimize instruction count (~1.3k vs 4.1k) and keep engines
balanced:
  * x arrives host-relaid CHANNEL-MAJOR [C, H, W] fp16 -> big contiguous DMAs
    and no on-device transpose at all.
  * Separable blur runs as a ping-pong cascade of large DVE adds over row
    bands ([1,3,3,1] = [1,1]^3 -> 3 adds per direction).
  * Conv runs WEIGHTS-STATIONARY: psum[oc128, 4 rows, 128 cols] accumulates
    9 taps of matmul(lhsT=w_tap[C,oc128], rhs=z[C, rows::2, cols::2]) with
    N=512 moving columns (the PE column floor: 576 matmuls/image).
  * Bias is folded into the PSUM evacuation on the scalar engine
    (activation Identity + per-partition bias).
  * Output is written channel-major [OC, OH, OW] and transposed on host.
"""
import sys

if "/opt/trn_rl_repo" not in sys.path:
    sys.path.insert(0, "/opt/trn_rl_repo")

import numpy as np

import concourse.bass as bass
import concourse.tile as tile
from concourse import bacc, mybir
from concourse.bass_utils import run_bass_kernel_spmd

F16 = mybir.dt.float16
F32 = mybir.dt.float32
F32R = mybir.dt.float32r

# "f32r": conv matmuls in float32r — self-loading (no InstLdweights), 1
# instruction per matmul, ~3.6x the per-matmul cost of fp16.
# "f16": conv matmuls in fp16 — 2 instructions per matmul (Ld+MM), fastest.
CONV_DTYPE = "f32r"

N_CORES = 8
C = 128
H = W = 256
OC = 256
OH = OW = 128
PITCH = 262          # fp16 row pitch (4B aligned); H1 writes cols [2,262)
XO = 4               # x col j at buffer col j+XO
NMAX = 36            # max band tile rows (2*16+4)

# out-row bands: small first/last band shortens pipeline fill/drain
BANDS = [(0, 4), (4, 8)] + [(12 + 16 * i, 16) for i in range(7)] + [(124, 4)]
assert BANDS[-1][0] + BANDS[-1][1] == OH

# fraction of each blur pass's rows offloaded to the (otherwise idle)
# GPSIMD engine; ~4.8x slower per element than DVE's 2x mode, so keep small
GPF = 0.20


def _build_bass(mode, repeat=1):
    nc = bacc.Bacc("TRN2", target_bir_lowering=False, debug=False)

    WDT = F32R if CONV_DTYPE == "f32r" else F16
    xc = nc.dram_tensor("xc", [C, H, W], F16, kind="ExternalInput").ap()
    w9 = nc.dram_tensor("w9", [C, 9, OC], WDT, kind="ExternalInput").ap()
    bias2 = nc.dram_tensor("bias2", [C, 2], F32, kind="ExternalInput").ap()
    if mode == "general":
        # kh[c,4] then kv[c,4] tap coefficients (broadcast per partition)
        kco = nc.dram_tensor("kco", [C, 8], F32, kind="ExternalInput").ap()
    out = nc.dram_tensor("out", [OC, OH, OW], F32, kind="ExternalOutput").ap()

    with tile.TileContext(nc) as tc:
        with (
            tc.tile_pool(name="const", bufs=1) as cpool,
            tc.tile_pool(name="apool", bufs=3) as apool,
            tc.tile_pool(name="bpool", bufs=2) as bpool,
            tc.tile_pool(name="zpool", bufs=2) as zpool,
            tc.tile_pool(name="osb", bufs=1) as opool,
            tc.tile_pool(name="ps", bufs=1, space=bass.MemorySpace.PSUM) as ppool,
        ):
            w_sb = cpool.tile([C, 9, OC], WDT)
            bias_sb = cpool.tile([C, 2], F32)
            if mode == "general":
                # needed by the very first blur op -> load up front
                kco_sb = cpool.tile([C, 8], F32, name="kco_sb")
                nc.sync.dma_start(kco_sb[:], kco[:])
            else:
                kco_sb = None

            def load_consts():
                # deferred so band-0/1 input DMAs lead the SP queue
                nc.sync.dma_start(w_sb[:], w9[:])
                nc.sync.dma_start(bias_sb[:], bias2[:])

            def blur_band(k):
                """Load + blur band k. Returns tile holding z:
                row t = y row (2*p0 + t), col j' = y col (j'-2) ... i.e.
                conv reads z rows (2r'+a), buffer cols (2s+b+2)."""
                p0, R = BANDS[k]
                n = 2 * R + 4
                lo = 2 * p0 - 2
                s_lo = max(0, -lo)
                s_hi = min(n, H - lo)

                A = apool.tile([C, NMAX, PITCH], F16, tag="A")
                B = bpool.tile([C, NMAX, PITCH], F16, tag="B")
                nc.sync.dma_start(
                    A[:, s_lo:s_hi, XO:XO + W],
                    xc[:, lo + s_lo:lo + s_hi, :],
                )
                # tiny border zeroes on DVE itself: they sit directly before
                # H1 in its FIFO (ACT's FIFO would couple them behind
                # PSUM-dependent evacs; GPSIMD's behind its blur shares)
                nc.vector.memset(A[:, 0:n, 0:XO], 0.0)
                nc.vector.memset(A[:, 0:n, XO + W:PITCH], 0.0)
                if s_lo > 0:
                    nc.vector.memset(A[:, 0:s_lo, XO:XO + W], 0.0)
                if s_hi < n:
                    nc.vector.memset(A[:, s_hi:n, XO:XO + W], 0.0)

                def finish(Zt):
                    return Zt

                def add(dst, nr, s0, rsh, c0, c1, csh):
                    """dst[:, 0:nr, c0:c1] = s0[:, rsh:rsh+nr, c0+csh:c1+csh]
                                           + s0[:, 0:nr, c0:c1] split DVE/GP."""
                    S = nr - max(1, int(round(GPF * nr))) if (GPF > 0 and R >= 8) else nr
                    nc.vector.tensor_add(
                        dst[:, 0:S, c0:c1],
                        s0[:, 0:S, c0:c1],
                        s0[:, rsh:rsh + S, c0 + csh:c1 + csh])
                    if S < nr:
                        nc.gpsimd.tensor_add(
                            dst[:, S:nr, c0:c1],
                            s0[:, S:nr, c0:c1],
                            s0[:, rsh + S:rsh + nr, c0 + csh:c1 + csh])

                if mode == "b1331":
                    # H: 3 shift-1 col adds, A->B->A->B
                    # u1 needed for y-cols <= 256 -> j in [-2,256] -> cols [2,261)
                    add(B, n, A, 0, 2, 261, 1)
                    add(A, n, B, 0, 2, 260, 1)
                    add(B, n, A, 0, 2, 259, 1)
                    # V: 3 shift-1 row adds, B->A->B->A
                    add(A, n - 1, B, 1, 2, 259, 0)
                    add(B, n - 2, A, 1, 2, 259, 0)
                    add(A, n - 3, B, 1, 2, 259, 0)
                    return finish(A)
                elif mode == "b1111":
                    # [1,1,1,1] = [1,1] * [1,0,1]: 2 adds per direction
                    add(B, n, A, 0, 2, 261, 1)
                    add(A, n, B, 0, 2, 259, 2)
                    add(B, n - 1, A, 1, 2, 259, 0)
                    add(A, n - 3, B, 2, 2, 259, 0)
                    return finish(A)
                else:
                    # general 4-tap: acc = sum_u k[u] * shift_u(x), per direction
                    stt = nc.vector.scalar_tensor_tensor
                    mul = mybir.AluOpType.mult
                    addop = mybir.AluOpType.add
                    nc.vector.tensor_scalar(
                        B[:, 0:n, 2:259], A[:, 0:n, 2:259], kco_sb[:, 0:1],
                        None, mul)
                    for u in range(1, 4):
                        stt(B[:, 0:n, 2:259], A[:, 0:n, 2 + u:259 + u],
                            kco_sb[:, u:u + 1], B[:, 0:n, 2:259], mul, addop)
                    nc.vector.tensor_scalar(
                        A[:, 0:n - 3, 2:259], B[:, 0:n - 3, 2:259], kco_sb[:, 4:5],
                        None, mul)
                    for u in range(1, 4):
                        stt(A[:, 0:n - 3, 2:259], B[:, u:n - 3 + u, 2:259],
                            kco_sb[:, 4 + u:5 + u], A[:, 0:n - 3, 2:259], mul, addop)
                    return finish(A)

            def finish_band(k, Zt):
                """Emit the f16->f32r convert for band k's blurred tile."""
                if CONV_DTYPE != "f32r":
                    return Zt, 2
                _, R = BANDS[k]
                n = 2 * R + 4
                # NMAX-2 rows: conv slices end at 8*(R//4-1)+a+8 <= 34
                # (last actually-read row is 32). Split so the first conv
                # chunk (reads rows <=16) can start after part a.
                Zr = zpool.tile([C, NMAX - 2, 258], F32R, tag="Zr")
                nz = n - 3
                if nz > 18:
                    nc.scalar.copy(Zr[:, 0:18, 0:257], Zt[:, 0:18, 2:259])
                    nc.scalar.copy(Zr[:, 18:nz, 0:257], Zt[:, 18:nz, 2:259])
                else:
                    nc.scalar.copy(Zr[:, 0:nz, 0:257], Zt[:, 0:nz, 2:259])
                return Zr, 0

            # out DRAM [OC=2*128, OH, OW] viewed per-partition as 2 oc chunks
            out4 = out.rearrange("(g p) r q -> p g r q", g=2)

            def conv_band(k, ZC, mid=None):
                Z, co = ZC
                p0, R = BANDS[k]
                ngrp = R // 4
                ot = opool.tile([C, 2, 16, OW], F32, tag="o")
                for half in range(2):
                    if half == 1 and mid is not None:
                        # emit the NEXT band's convert between the halves so
                        # it overlaps this band's second-half matmuls
                        mid()
                    # 8-row psum chunks (2 banks each): finer-grained evac ->
                    # next band's matmuls only wait their own chunk's evac
                    for gg in range(max(1, ngrp // 2)):
                        glo = 2 * gg
                        ghi = min(ngrp, glo + 2)
                        rr = 4 * (ghi - glo)
                        ps = ppool.tile([C, 8, OW], F32, tag=f"ps{half}{gg}")
                        for t in range(9):
                            a, b = divmod(t, 3)
                            lhsT = w_sb[:, t, 128 * half:128 * (half + 1)]
                            for g in range(glo, ghi):
                                nc.tensor.matmul(
                                    ps[:, 4 * (g - glo):4 * (g - glo) + 4, :],
                                    lhsT,
                                    Z[:, 8 * g + a:8 * g + a + 8:2,
                                      b + co:b + co + 256:2],
                                    start=(t == 0), stop=(t == 8),
                                    skip_group_check=True,
                                )
                        nc.scalar.activation(
                            ot[:, half, 8 * gg:8 * gg + rr, :], ps[:, 0:rr, :],
                            mybir.ActivationFunctionType.Identity,
                            bias=bias_sb[:, half:half + 1],
                        )
                nc.scalar.dma_start(
                    out4[:, :, p0:p0 + R, :], ot[:, :, 0:R, :],
                )

            for rep in range(repeat):
                # conv(k-1) emitted BEFORE blur(k)'s DVE chain completes;
                # band k's convert is emitted between conv(k-1)'s two halves
                # so ACT runs [evac(k-1,h0), cvt(k), evac(k-1,h1)] and the
                # PE never waits a full convert between bands
                raw = blur_band(0)
                if rep == 0:
                    load_consts()
                prev_zc = finish_band(0, raw)
                for k in range(1, len(BANDS)):
                    raw = blur_band(k)
                    holder = []
                    conv_band(k - 1, prev_zc,
                              mid=lambda kk=k, rr=raw: holder.append(
                                  finish_band(kk, rr)))
                    prev_zc = holder[0]
                conv_band(len(BANDS) - 1, prev_zc)

    nc.compile()
    return nc


_NC = {}


def _get_nc(mode="b1331", repeat=1):
    key = (mode, repeat)
    if key not in _NC:
        _NC[key] = _build_bass(mode, repeat)
    return _NC[key]


def _blur_mode(bk):
    k8 = bk / bk.sum() * 8.0
    if np.allclose(k8, [1.0, 3.0, 3.0, 1.0], rtol=1e-6, atol=1e-7):
        return "b1331"
    k4 = bk / bk.sum() * 4.0
    if np.allclose(k4, [1.0, 1.0, 1.0, 1.0], rtol=1e-6, atol=1e-7):
        return "b1111"
    return "general"


def _prepare_in_maps(x, conv_w, conv_b, blur_kernel):
    x = np.asarray(x, dtype=np.float32)
    conv_w = np.asarray(conv_w, dtype=np.float32)
    conv_b = np.asarray(conv_b, dtype=np.float32)
    bk = np.asarray(blur_kernel, dtype=np.float32)

    mode = _blur_mode(bk)
    if mode in ("b1331", "b1111"):
        # device cascade computes the unnormalized integer-tap blur;
        # fold the 2D normalization into the conv weights
        wscale = 1.0 / (bk.sum() ** 2)
    else:
        wscale = 1.0  # normalized taps shipped via kco

    wdt = np.float32 if CONV_DTYPE == "f32r" else np.float16
    w9 = np.ascontiguousarray(
        (conv_w * wscale).reshape(9, C, OC).transpose(1, 0, 2).astype(wdt)
    )
    bias2 = np.ascontiguousarray(conv_b.reshape(2, 128).T.astype(np.float32))

    base = {"w9": w9, "bias2": bias2}
    if mode == "general":
        k1 = (bk / bk.sum()).astype(np.float32)
        kco = np.broadcast_to(
            np.concatenate([k1, k1])[None, :], (C, 8)
        ).astype(np.float32)
        base["kco"] = np.ascontiguousarray(kco)

    in_maps = []
    for i in range(N_CORES):
        im = dict(base)
        im["xc"] = np.ascontiguousarray(x[i].transpose(2, 0, 1).astype(np.float16))
        in_maps.append(im)
    return mode, in_maps


def _run(mode, in_maps, **kwargs):
    nc = _get_nc(mode)
    return run_bass_kernel_spmd(nc, in_maps, core_ids=list(range(N_CORES)), **kwargs)


def kernel(x, conv_w, conv_b, blur_kernel):
    mode, in_maps = _prepare_in_maps(x, conv_w, conv_b, blur_kernel)
    res = _run(mode, in_maps)
    # device output is channel-major [OC, OH, OW] -> NHWC
    return np.stack(
        [res.results[i]["out"].transpose(1, 2, 0) for i in range(N_CORES)], axis=0
    )


# revision 3
# speedup vs baseline: 265.2823x; 233.2015x over previous
"""ConvDownsample2D (StyleGAN2 FIR blur + strided conv) for 8 Trainium2 cores.

Sharding: data-parallel over batch, 1 image per NeuronCore.

v2 design — minimize instruction count (~1.3k vs 4.1k) and keep engines
balanced:
  * x arrives host-relaid CHANNEL-MAJOR [C, H, W] fp16 -> big contiguous DMAs
    and no on-device transpose at all.
  * Separable blur runs as a ping-pong cascade of large DVE adds over row
    bands ([1,3,3,1] = [1,1]^3 -> 3 adds per direction).
  * Conv runs WEIGHTS-STATIONARY: psum[oc128, 4 rows, 128 cols] accumulates
    9 taps of matmul(lhsT=w_tap[C,oc128], rhs=z[C, rows::2, cols::2]) with
    N=512 moving columns (the PE column floor: 576 matmuls/image).
  * Bias is folded into the PSUM evacuation on the scalar engine
    (activation Identity + per-partition bias).
  * Output is written channel-major [OC, OH, OW] and transposed on host.
"""
import sys

if "/opt/trn_rl_repo" not in sys.path:
    sys.path.insert(0, "/opt/trn_rl_repo")

import numpy as np

import concourse.bass as bass
import concourse.tile as tile
from concourse import bacc, mybir
from concourse.bass_utils import run_bass_kernel_spmd

F16 = mybir.dt.float16
F32 = mybir.dt.float32
F32R = mybir.dt.float32r

# "f32r": conv matmuls in float32r — self-loading (no InstLdweights), 1
# instruction per matmul, ~3.6x the per-matmul cost of fp16.
# "f16": conv matmuls in fp16 — 2 instructions per matmul (Ld+MM), fastest.
CONV_DTYPE = "f32r"

N_CORES = 8
C = 128
H = W = 256
OC = 256
OH = OW = 128
PITCH = 264          # fp16 row pitch; borders [0:4)+[260:264) are zeroed
XO = 4               # x col j at buffer col j+XO
NMAX = 36            # max band tile rows (2*16+4)

# out-row bands: small first/last band shortens pipeline fill/drain
BANDS = [(0, 4), (4, 8)] + [(12 + 16 * i, 16) for i in range(7)] + [(124, 4)]
assert BANDS[-1][0] + BANDS[-1][1] == OH

# fraction of each blur pass's rows offloaded to the (otherwise idle)
# GPSIMD engine; ~4.8x slower per element than DVE's 2x mode, so keep small
GPF = 0.20


def _build_bass(mode, repeat=1):
    nc = bacc.Bacc("TRN2", target_bir_lowering=False, debug=False)

    WDT = F32R if CONV_DTYPE == "f32r" else F16
    xc = nc.dram_tensor("xc", [C, H, W], F16, kind="ExternalInput").ap()
    w9 = nc.dram_tensor("w9", [C, 9, OC], WDT, kind="ExternalInput").ap()
    bias2 = nc.dram_tensor("bias2", [C, 2], F32, kind="ExternalInput").ap()
    if mode == "general":
        # kh[c,4] then kv[c,4] tap coefficients (broadcast per partition)
        kco = nc.dram_tensor("kco", [C, 8], F32, kind="ExternalInput").ap()
    out = nc.dram_tensor("out", [OC, OH, OW], F32, kind="ExternalOutput").ap()

    with tile.TileContext(nc) as tc:
        with (
            tc.tile_pool(name="const", bufs=1) as cpool,
            tc.tile_pool(name="apool", bufs=3) as apool,
            tc.tile_pool(name="bpool", bufs=2) as bpool,
            tc.tile_pool(name="zpool", bufs=2) as zpool,
            tc.tile_pool(name="osb", bufs=1) as opool,
            tc.tile_pool(name="ps", bufs=1, space=bass.MemorySpace.PSUM) as ppool,
        ):
            w_sb = cpool.tile([C, 9, OC], WDT)
            bias_sb = cpool.tile([C, 2], F32)
            if mode == "general":
                # needed by the very first blur op -> load up front
                kco_sb = cpool.tile([C, 8], F32, name="kco_sb")
                nc.sync.dma_start(kco_sb[:], kco[:])
            else:
                kco_sb = None

            def load_consts():
                # deferred so band-0/1 input DMAs lead the SP queue
                nc.sync.dma_start(w_sb[:], w9[:])
                nc.sync.dma_start(bias_sb[:], bias2[:])

            def blur_band(k):
                """Load + blur band k. Returns tile holding z:
                row t = y row (2*p0 + t), col j' = y col (j'-2) ... i.e.
                conv reads z rows (2r'+a), buffer cols (2s+b+2)."""
                p0, R = BANDS[k]
                n = 2 * R + 4
                lo = 2 * p0 - 2
                s_lo = max(0, -lo)
                s_hi = min(n, H - lo)

                A = apool.tile([C, NMAX, PITCH], F16, tag="A")
                B = bpool.tile([C, NMAX, PITCH], F16, tag="B")
                nc.sync.dma_start(
                    A[:, s_lo:s_hi, XO:XO + W],
                    xc[:, lo + s_lo:lo + s_hi, :],
                )
                # tiny border zeroes on DVE itself: they sit directly before
                # H1 in its FIFO, and their narrow column ranges don't
                # overlap the DMA region (a merged strided AP would span the
                # full row and serialize behind the input DMA)
                nc.vector.memset(A[:, 0:n, 0:XO], 0.0)
                nc.vector.memset(A[:, 0:n, XO + W:PITCH], 0.0)
                if s_lo > 0:
                    nc.vector.memset(A[:, 0:s_lo, XO:XO + W], 0.0)
                if s_hi < n:
                    nc.vector.memset(A[:, s_hi:n, XO:XO + W], 0.0)

                def finish(Zt):
                    return Zt

                def add(dst, nr, s0, rsh, c0, c1, csh):
                    """dst[:, 0:nr, c0:c1] = s0[:, rsh:rsh+nr, c0+csh:c1+csh]
                                           + s0[:, 0:nr, c0:c1] split DVE/GP."""
                    S = nr - max(1, int(round(GPF * nr))) if (GPF > 0 and R >= 8) else nr
                    nc.vector.tensor_add(
                        dst[:, 0:S, c0:c1],
                        s0[:, 0:S, c0:c1],
                        s0[:, rsh:rsh + S, c0 + csh:c1 + csh])
                    if S < nr:
                        nc.gpsimd.tensor_add(
                            dst[:, S:nr, c0:c1],
                            s0[:, S:nr, c0:c1],
                            s0[:, rsh + S:rsh + nr, c0 + csh:c1 + csh])

                if mode == "b1331":
                    # H: 3 shift-1 col adds, A->B->A->B
                    # u1 needed for y-cols <= 256 -> j in [-2,256] -> cols [2,261)
                    add(B, n, A, 0, 2, 261, 1)
                    add(A, n, B, 0, 2, 260, 1)
                    add(B, n, A, 0, 2, 259, 1)
                    # V: 3 shift-1 row adds, B->A->B->A
                    add(A, n - 1, B, 1, 2, 259, 0)
                    add(B, n - 2, A, 1, 2, 259, 0)
                    add(A, n - 3, B, 1, 2, 259, 0)
                    return finish(A)
                elif mode == "b1111":
                    # [1,1,1,1] = [1,1] * [1,0,1]: 2 adds per direction
                    add(B, n, A, 0, 2, 261, 1)
                    add(A, n, B, 0, 2, 259, 2)
                    add(B, n - 1, A, 1, 2, 259, 0)
                    add(A, n - 3, B, 2, 2, 259, 0)
                    return finish(A)
                else:
                    # general 4-tap: acc = sum_u k[u] * shift_u(x), per direction
                    stt = nc.vector.scalar_tensor_tensor
                    mul = mybir.AluOpType.mult
                    addop = mybir.AluOpType.add
                    nc.vector.tensor_scalar(
                        B[:, 0:n, 2:259], A[:, 0:n, 2:259], kco_sb[:, 0:1],
                        None, mul)
                    for u in range(1, 4):
                        stt(B[:, 0:n, 2:259], A[:, 0:n, 2 + u:259 + u],
                            kco_sb[:, u:u + 1], B[:, 0:n, 2:259], mul, addop)
                    nc.vector.tensor_scalar(
                        A[:, 0:n - 3, 2:259], B[:, 0:n - 3, 2:259], kco_sb[:, 4:5],
                        None, mul)
                    for u in range(1, 4):
                        stt(A[:, 0:n - 3, 2:259], B[:, u:n - 3 + u, 2:259],
                            kco_sb[:, 4 + u:5 + u], A[:, 0:n - 3, 2:259], mul, addop)
                    return finish(A)

            def finish_band(k, Zt):
                """Emit the f16->f32r convert for band k's blurred tile."""
                if CONV_DTYPE != "f32r":
                    return Zt, 2
                _, R = BANDS[k]
                n = 2 * R + 4
                # NMAX-2 rows: conv slices end at 8*(R//4-1)+a+8 <= 34
                # (last actually-read row is 32). Split so the first conv
                # chunk (reads rows <=16) can start after part a.
                Zr = zpool.tile([C, NMAX - 2, 258], F32R, tag="Zr")
                nz = n - 3
                if nz > 18:
                    nc.scalar.copy(Zr[:, 0:18, 0:257], Zt[:, 0:18, 2:259])
                    nc.scalar.copy(Zr[:, 18:nz, 0:257], Zt[:, 18:nz, 2:259])
                else:
                    nc.scalar.copy(Zr[:, 0:nz, 0:257], Zt[:, 0:nz, 2:259])
                return Zr, 0

            # out DRAM [OC=2*128, OH, OW] viewed per-partition as 2 oc chunks
            out4 = out.rearrange("(g p) r q -> p g r q", g=2)

            def conv_band(k, ZC, mid=None):
                Z, co = ZC
                p0, R = BANDS[k]
                assert R in (4, 8, 16), "psum chunking assumes 1/2/4 groups"
                ngrp = R // 4
                ot = opool.tile([C, 2, 16, OW], F32, tag="o")
                for half in range(2):
                    if half == 1 and mid is not None:
                        # emit the NEXT band's convert between the halves so
                        # it overlaps this band's second-half matmuls
                        mid()
                    # one 4-bank psum tile per half; its evac runs during the
                    # other half's matmuls, so the WAR on the next band's
                    # matmuls is released in time
                    ps = ppool.tile([C, 16, OW], F32, tag=f"ps{half}")
                    for t in range(9):
                        a, b = divmod(t, 3)
                        lhsT = w_sb[:, t, 128 * half:128 * (half + 1)]
                        for g in range(ngrp):
                            nc.tensor.matmul(
                                ps[:, 4 * g:4 * g + 4, :],
                                lhsT,
                                Z[:, 8 * g + a:8 * g + a + 8:2,
                                  b + co:b + co + 256:2],
                                start=(t == 0), stop=(t == 8),
                                skip_group_check=True,
                            )
                    nc.scalar.activation(
                        ot[:, half, 0:R, :], ps[:, 0:R, :],
                        mybir.ActivationFunctionType.Identity,
                        bias=bias_sb[:, half:half + 1],
                    )
                nc.scalar.dma_start(
                    out4[:, :, p0:p0 + R, :], ot[:, :, 0:R, :],
                )

            for rep in range(repeat):
                # conv(k-1) emitted BEFORE blur(k)'s DVE chain completes;
                # band k's convert is emitted between conv(k-1)'s two halves
                # so ACT runs [evac(k-1,h0), cvt(k), evac(k-1,h1)] and the
                # PE never waits a full convert between bands
                raw = blur_band(0)
                if rep == 0:
                    load_consts()
                prev_zc = finish_band(0, raw)
                for k in range(1, len(BANDS)):
                    raw = blur_band(k)
                    holder = []
                    conv_band(k - 1, prev_zc,
                              mid=lambda kk=k, rr=raw: holder.append(
                                  finish_band(kk, rr)))
                    prev_zc = holder[0]
                conv_band(len(BANDS) - 1, prev_zc)

    nc.compile()
    return nc


_NC = {}


def _get_nc(mode="b1331", repeat=1):
    key = (mode, repeat)
    if key not in _NC:
        _NC[key] = _build_bass(mode, repeat)
    return _NC[key]


def _blur_mode(bk):
    k8 = bk / bk.sum() * 8.0
    if np.allclose(k8, [1.0, 3.0, 3.0, 1.0], rtol=1e-6, atol=1e-7):
        return "b1331"
    k4 = bk / bk.sum() * 4.0
    if np.allclose(k4, [1.0, 1.0, 1.0, 1.0], rtol=1e-6, atol=1e-7):
        return "b1111"
    return "general"


def _prepare_in_maps(x, conv_w, conv_b, blur_kernel):
    x = np.asarray(x, dtype=np.float32)
    conv_w = np.asarray(conv_w, dtype=np.float32)
    conv_b = np.asarray(conv_b, dtype=np.float32)
    bk = np.asarray(blur_kernel, dtype=np.float32)

    mode = _blur_mode(bk)
    if mode in ("b1331", "b1111"):
        # device cascade computes the unnormalized integer-tap blur;
        # fold the 2D normalization into the conv weights
        wscale = 1.0 / (bk.sum() ** 2)
    else:
        wscale = 1.0  # normalized taps shipped via kco

    wdt = np.float32 if CONV_DTYPE == "f32r" else np.float16
    w9 = np.ascontiguousarray(
        (conv_w * wscale).reshape(9, C, OC).transpose(1, 0, 2).astype(wdt)
    )
    bias2 = np.ascontiguousarray(conv_b.reshape(2, 128).T.astype(np.float32))

    base = {"w9": w9, "bias2": bias2}
    if mode == "general":
        k1 = (bk / bk.sum()).astype(np.float32)
        kco = np.broadcast_to(
            np.concatenate([k1, k1])[None, :], (C, 8)
        ).astype(np.float32)
        base["kco"] = np.ascontiguousarray(kco)

    in_maps = []
    for i in range(N_CORES):
        im = dict(base)
        im["xc"] = np.ascontiguousarray(x[i].transpose(2, 0, 1).astype(np.float16))
        in_maps.append(im)
    return mode, in_maps


def _run(mode, in_maps, **kwargs):
    nc = _get_nc(mode)
    return run_bass_kernel_spmd(nc, in_maps, core_ids=list(range(N_CORES)), **kwargs)


def kernel(x, conv_w, conv_b, blur_kernel):
    mode, in_maps = _prepare_in_maps(x, conv_w, conv_b, blur_kernel)
    res = _run(mode, in_maps)
    # device output is channel-major [OC, OH, OW] -> NHWC
    return np.stack(
        [res.results[i]["out"].transpose(1, 2, 0) for i in range(N_CORES)], axis=0
    )
